# revision 3
# baseline (speedup 1.0000x reference)
"""Expert-parallel MoE layer for 8 Trainium2 NeuronCores.

Strategy: each of the 8 experts is assigned to one core. The host computes
the routing (which tokens go to which expert and with what combined weight),
gathers + transposes each expert's tokens into a padded [D, C] activation
matrix in bf16, and each core runs a fused  gelu(x @ W1 + b1) @ W2  kernel
for its expert. The host applies the per-token combine weight and the
(zero-ish) b2 term while scatter-adding the per-expert outputs back into the
full [B, S, D] output, so neither cw nor b2 ever travels to the device.

All matmul operands and the output travel as bf16 (norm rel err ~4e-3,
budget 2e-2): halves DMA bytes and SBUF pressure vs f32r at the same
1 column/cycle PE rate.

Schedule: token columns are processed in 512-col sub-blocks, software-
pipelined as L1(s0), L1(s1), L2(s0), L1(s2), L2(s1), ... so expert outputs
start draining early. Inputs are fed through both HWDGE rings in parallel
(sync: weights + first sub; scalar: remaining early x^T + b1) with late x^T
tiles on the gpsimd SWDGE ring, so the PE never starves on the first fc
sweep. Output DMAs ride the scalar ring between gelu batches.
"""

import sys

if "/opt/trn_rl_repo" not in sys.path:
    sys.path.insert(0, "/opt/trn_rl_repo")

import ml_dtypes
import numpy as np

import concourse.bass as bass
import concourse.tile as tile
from concourse import bacc, mybir
from concourse.bass_utils import run_bass_kernel_spmd

B, S, D, F, E, TOPK = 4, 2048, 512, 1024, 8, 2
T = B * S
F32 = mybir.dt.float32
BF16 = mybir.dt.bfloat16
NPBF16 = ml_dtypes.bfloat16

DC = D // 128  # 4 contraction chunks for x @ W1
FC = F // 128  # 8 contraction chunks for h @ W2

# PE clock-ramp warmup: number of 128-col dummy matmuls issued before the
# first data-dependent matmul (tuned against the trace so the chain ends
# right as the first x^T/W1 slices land).
N_WARMUP = 30

# Set by test harness to capture a profile; harness-invisible otherwise.
TRACE = False
LAST_RESULTS = None

_nc_cache = {}


def _sub_blocks(C):
    """Split C token columns into sub-block sizes (<=512, one PSUM bank).

    The final two sub-blocks are halved so the kernel tail (last L2 group ->
    cast -> output DMA) drains a short block instead of a full 512.
    """
    assert C % 64 == 0 and C >= 512
    subs = []
    rem = C
    while rem > 512 + 256:
        subs.append(512)
        rem -= 512
    while rem > 256:
        subs.append(256)
        rem -= 256
    if rem:
        subs.append(rem)
    assert sum(subs) == C
    return subs


def _build_nc(C):
    nc = bacc.Bacc("TRN2", num_devices=E)

    xt = nc.dram_tensor("xt", [D, C], BF16, kind="ExternalInput")
    w1 = nc.dram_tensor("w1", [D, F], BF16, kind="ExternalInput")
    b1 = nc.dram_tensor("b1", [F, 1], F32, kind="ExternalInput")
    w2 = nc.dram_tensor("w2", [F, D], BF16, kind="ExternalInput")
    yt = nc.dram_tensor("yt", [D, C], BF16, kind="ExternalOutput")

    # DRAM views with the 128-partition chunk dim split out
    xt_r = xt.rearrange("(c p) t -> p c t", p=128)  # [128, DC, C]
    w1_r = w1.rearrange("(c p) f -> p c f", p=128)  # [128, DC, F]
    w2_r = w2.rearrange("(c p) d -> p c d", p=128)  # [128, FC, D]
    b1_r = b1.rearrange("(c p) o -> p (c o)", p=128)  # [128, FC]
    yt_r = yt.rearrange("(c p) t -> p c t", p=128)  # [128, DC, C]

    subs = _sub_blocks(C)
    offs = []
    off = 0
    for s in subs:
        offs.append(off)
        off += s
    nsub = len(subs)

    with tile.TileContext(nc) as tc:
        with (
            tc.tile_pool(name="consts", bufs=1) as consts,
            tc.tile_pool(name="xtp", bufs=1) as xtp,
            tc.tile_pool(name="hp", bufs=18) as hp,
            tc.tile_pool(name="ybig", bufs=8) as ybigp,
            tc.tile_pool(name="ps_h", bufs=4, space="PSUM") as ps_h,
            tc.tile_pool(name="ps_y", bufs=4, space="PSUM") as ps_y,
        ):
            # PE warmup: the clock gate keeps the PE at a low p-state until
            # it has been busy a few us. These dummy matmuls (zero operands,
            # no DMA deps) warm the clock during the DMA wait for free;
            # 128-col granularity so the handoff to real work is tight.
            wu_w = consts.tile([128, 128], BF16, tag="wu_w")
            nc.vector.memset(wu_w[:, :], 0.0)
            wu_x = consts.tile([128, 128], BF16, tag="wu_x")
            nc.vector.memset(wu_x[:, :], 0.0)
            wu_ps = ps_h.tile([128, 128], F32, tag="psh")
            for k in range(N_WARMUP):
                nc.tensor.matmul(
                    wu_ps[:, :], wu_w[:, :], wu_x[:, :],
                    start=(k == 0), stop=(k == N_WARMUP - 1),
                )

            # ---- sync HWDGE ring: w1 (staged), first x^T half, w2 ----
            # w1 first slice = only the f-columns the first fc group needs,
            # so the first matmul gates on 128KB.
            w1_sb = consts.tile([128, DC, F], BF16, tag="w1")
            nc.sync.dma_start(out=w1_sb[:, :, 0:128], in_=w1_r[:, :, 0:128])

            # first sub-block's dc 0-1 half; dc 2-3 ride the scalar ring in
            # parallel so the first fc group's 4-matmul chain gates on two
            # 256KB transfers that overlap across rings.
            xt00a = xtp.tile([128, 2, 512], BF16, tag="xt0a", name="xt_sb0a")
            nc.sync.dma_start(out=xt00a[:, :, :], in_=xt_r[:, 0:2, 0:subs[0]])

            # rest of w1, staged so each slice lands ahead of its fc group
            for lo, hi in ((128, 512), (512, 1024)):
                nc.sync.dma_start(out=w1_sb[:, :, lo:hi], in_=w1_r[:, :, lo:hi])

            # w2: first d-chunk unblocks L2(s0) (needed ~early with the
            # pipelined schedule), then the rest
            w2_sb = consts.tile([128, FC, D], BF16, tag="w2")
            nc.sync.dma_start(out=w2_sb[:, :, 0:128], in_=w2_r[:, :, 0:128])
            nc.sync.dma_start(out=w2_sb[:, :, 128:D], in_=w2_r[:, :, 128:D])

            # ---- scalar HWDGE ring: first x^T other half, subs 1-2, b1 ----
            xt00b = xtp.tile([128, 2, 512], BF16, tag="xt0b", name="xt_sb0b")
            nc.scalar.dma_start(out=xt00b[:, :, :], in_=xt_r[:, 2:4, 0:subs[0]])

            xt_sub = {}

            def _load_xt(si, eng):
                soff, ssz = offs[si], subs[si]
                t = xtp.tile([128, DC, ssz], BF16, tag=f"xt{si}", name=f"xt_sb{si}")
                eng.dma_start(out=t[:, :, :], in_=xt_r[:, :, soff:soff + ssz])
                xt_sub[si] = t

            for si in (1, 2):
                if si < nsub:
                    _load_xt(si, nc.scalar)

            b1_sb = consts.tile([128, FC], F32, tag="b1")
            nc.scalar.dma_start(out=b1_sb[:, :], in_=b1_r[:, :])

            # ---- gpsimd SWDGE ring: late x^T tiles (needed >30us in) ----
            for si in range(3, nsub):
                _load_xt(si, nc.gpsimd)

            def layer1(si):
                ssz = subs[si]
                for fc in range(FC):
                    ps = ps_h.tile([128, ssz], F32, tag="psh")
                    for dc in range(DC):
                        if si == 0:
                            rhs = (xt00a[:, dc, 0:ssz] if dc < 2
                                   else xt00b[:, dc - 2, 0:ssz])
                        else:
                            rhs = xt_sub[si][:, dc, :]
                        nc.tensor.matmul(
                            ps[:, :],
                            w1_sb[:, dc, fc * 128:(fc + 1) * 128],
                            rhs,
                            start=(dc == 0),
                            stop=(dc == DC - 1),
                        )
                    h = hp.tile([128, ssz], BF16, tag="h")
                    nc.scalar.activation(
                        h[:, :], ps[:, :],
                        mybir.ActivationFunctionType.Gelu_apprx_tanh,
                        bias=b1_sb[:, fc:fc + 1],
                    )
                    h_tiles[(si, fc)] = h

            def layer2(si):
                soff, ssz = offs[si], subs[si]
                for dc in range(DC):
                    ps2 = ps_y.tile([128, ssz], F32, tag="psy")
                    for fc in range(FC):
                        nc.tensor.matmul(
                            ps2[:, :],
                            w2_sb[:, fc, dc * 128:(dc + 1) * 128],
                            h_tiles[(si, fc)][:, :],
                            start=(fc == 0),
                            stop=(fc == FC - 1),
                        )
                    # PSUM -> bf16 SBUF on the DVE; combine weights and b2
                    # are applied on the host during the scatter.
                    yout = ybigp.tile([128, ssz], BF16, tag="yout")
                    nc.vector.tensor_copy(out=yout[:, :], in_=ps2[:, :])
                    nc.scalar.dma_start(
                        out=yt_r[:, dc, soff:soff + ssz],
                        in_=yout[:, :],
                    )

            # software pipeline: L1 runs one sub ahead of L2, so outputs
            # drain throughout instead of bunching at the end
            h_tiles = {}
            layer1(0)
            for si in range(1, nsub):
                layer1(si)
                layer2(si - 1)
            layer2(nsub - 1)

    nc.finalize()
    return nc


def kernel(hidden, top_k_indices, top_k_weights, W1, b1, W2, b2):
    global LAST_RESULTS
    x = np.ascontiguousarray(np.asarray(hidden, dtype=np.float32).reshape(T, D))
    idx = np.asarray(top_k_indices).reshape(T, TOPK)
    w = np.asarray(top_k_weights, dtype=np.float32).reshape(T, TOPK)
    W1 = np.asarray(W1, dtype=np.float32)
    b1 = np.asarray(b1, dtype=np.float32)
    W2 = np.asarray(W2, dtype=np.float32)
    b2 = np.asarray(b2, dtype=np.float32)

    # Host routing: token lists + combined weights per expert
    tok_lists, cw_lists = [], []
    for e in range(E):
        m = idx == e
        toks = np.nonzero(m.any(axis=1))[0]
        cw_t = (w * m).sum(axis=1)[toks]
        tok_lists.append(toks)
        cw_lists.append(cw_t)

    maxn = max(len(t) for t in tok_lists)
    C = max(512, -(-maxn // 64) * 64)

    if C not in _nc_cache:
        _nc_cache[C] = _build_nc(C)
    nc = _nc_cache[C]

    in_maps = []
    for e in range(E):
        toks = tok_lists[e]
        n = len(toks)
        xt = np.zeros((D, C), NPBF16)
        xt[:, :n] = x[toks].T.astype(NPBF16)
        in_maps.append({
            "xt": xt,
            "w1": np.ascontiguousarray(W1[e].astype(NPBF16)),
            "b1": np.ascontiguousarray(b1[e].reshape(F, 1)),
            "w2": np.ascontiguousarray(W2[e].astype(NPBF16)),
        })

    kwargs = {}
    if TRACE:
        kwargs = dict(trace=True, trace_cores=list(range(E)))
    res = run_bass_kernel_spmd(nc, in_maps, core_ids=list(range(E)), **kwargs)
    LAST_RESULTS = res

    out = np.zeros((T, D), np.float32)
    for e in range(E):
        toks = tok_lists[e]
        n = len(toks)
        y = res.results[e]["yt"][:, :n].astype(np.float32).T
        out[toks] += cw_lists[e][:, None] * y
        if b2[e].any():
            out[toks] += cw_lists[e][:, None] * b2[e][None, :]
    return out.reshape(B, S, D)


# revision 6
# speedup vs baseline: 1.0824x; 1.0824x over previous
"""Expert-parallel MoE layer for 8 Trainium2 NeuronCores.

Strategy: each of the 8 experts is assigned to one core. The host computes
the routing (which tokens go to which expert and with what combined weight),
gathers + transposes each expert's tokens into a padded [D, C] activation
matrix in bf16, and each core runs a fused  gelu(x @ W1 + b1) @ W2  kernel
for its expert. The host applies the per-token combine weight and the
(zero-ish) b2 term while scatter-adding the per-expert outputs back into the
full [B, S, D] output, so neither cw nor b2 ever travels to the device.

All matmul operands and the output travel as bf16 (norm rel err ~4e-3,
budget 2e-2): halves DMA bytes and SBUF pressure vs f32r at the same
1 column/cycle PE rate.

Schedule: token columns are processed in 512-col sub-blocks, software-
pipelined as L1(s0), L1(s1), L2(s0), L1(s2), L2(s1), ... so expert outputs
start draining early. Inputs are fed through both HWDGE rings in parallel
(sync: weights + first sub; scalar: remaining early x^T + b1) with late x^T
tiles on the gpsimd SWDGE ring, so the PE never starves on the first fc
sweep. Output DMAs ride the scalar ring between gelu batches.
"""

import sys

if "/opt/trn_rl_repo" not in sys.path:
    sys.path.insert(0, "/opt/trn_rl_repo")

import ml_dtypes
import numpy as np

import concourse.bass as bass
import concourse.tile as tile
from concourse import bacc, mybir
from concourse.bass_utils import run_bass_kernel_spmd

B, S, D, F, E, TOPK = 4, 2048, 512, 1024, 8, 2
T = B * S
F32 = mybir.dt.float32
BF16 = mybir.dt.bfloat16
NPBF16 = ml_dtypes.bfloat16

DC = D // 128  # 4 contraction chunks for x @ W1
FC = F // 128  # 8 contraction chunks for h @ W2

# PE clock-ramp warmup: number of 128-col dummy matmuls issued before the
# first data-dependent matmul (tuned against the trace so the chain ends
# right as the first x^T/W1 slices land).
N_WARMUP = 36

# Set by test harness to capture a profile; harness-invisible otherwise.
TRACE = False
LAST_RESULTS = None

_nc_cache = {}


def _sub_blocks(C):
    """Split C token columns into sub-block sizes (<=512, one PSUM bank)."""
    assert C % 64 == 0 and C >= 512
    subs = []
    rem = C
    while rem > 0:
        s = min(512, rem)
        subs.append(s)
        rem -= s
    assert sum(subs) == C
    return subs


def _build_nc(C):
    nc = bacc.Bacc("TRN2", num_devices=E)

    xt = nc.dram_tensor("xt", [D, C], BF16, kind="ExternalInput")
    w1 = nc.dram_tensor("w1", [D, F], BF16, kind="ExternalInput")
    b1 = nc.dram_tensor("b1", [F, 1], F32, kind="ExternalInput")
    w2 = nc.dram_tensor("w2", [F, D], BF16, kind="ExternalInput")
    yt = nc.dram_tensor("yt", [D, C], BF16, kind="ExternalOutput")

    # DRAM views with the 128-partition chunk dim split out
    xt_r = xt.rearrange("(c p) t -> p c t", p=128)  # [128, DC, C]
    w1_r = w1.rearrange("(c p) f -> p c f", p=128)  # [128, DC, F]
    w2_r = w2.rearrange("(c p) d -> p c d", p=128)  # [128, FC, D]
    b1_r = b1.rearrange("(c p) o -> p (c o)", p=128)  # [128, FC]
    yt_r = yt.rearrange("(c p) t -> p c t", p=128)  # [128, DC, C]

    subs = _sub_blocks(C)
    offs = []
    off = 0
    for s in subs:
        offs.append(off)
        off += s
    nsub = len(subs)

    with tile.TileContext(nc) as tc:
        with (
            tc.tile_pool(name="consts", bufs=1) as consts,
            tc.tile_pool(name="xtp", bufs=1) as xtp,
            tc.tile_pool(name="hp", bufs=18) as hp,
            tc.tile_pool(name="ybig", bufs=8) as ybigp,
            tc.tile_pool(name="ps_h", bufs=4, space="PSUM") as ps_h,
            tc.tile_pool(name="ps_y", bufs=4, space="PSUM") as ps_y,
        ):
            # PE warmup: the clock gate keeps the PE at a low p-state until
            # it has been busy a few us. These dummy matmuls (zero operands,
            # no DMA deps) warm the clock during the DMA wait for free;
            # 128-col granularity so the handoff to real work is tight.
            wu_w = consts.tile([128, 128], BF16, tag="wu_w")
            nc.vector.memset(wu_w[:, :], 0.0)
            wu_x = consts.tile([128, 128], BF16, tag="wu_x")
            nc.vector.memset(wu_x[:, :], 0.0)
            wu_ps = ps_h.tile([128, 128], F32, tag="psh")
            for k in range(N_WARMUP):
                nc.tensor.matmul(
                    wu_ps[:, :], wu_w[:, :], wu_x[:, :],
                    start=(k == 0), stop=(k == N_WARMUP - 1),
                )

            # ---- sync HWDGE ring: w1 (staged), first x^T half, w2 ----
            # w1 first slice = only the f-columns the first fc group needs,
            # so the first matmul gates on 128KB.
            w1_sb = consts.tile([128, DC, F], BF16, tag="w1")
            nc.sync.dma_start(out=w1_sb[:, :, 0:128], in_=w1_r[:, :, 0:128])

            # first sub-block's dc 0-1 half; dc 2-3 ride the scalar ring in
            # parallel so the first fc group's 4-matmul chain gates on two
            # 256KB transfers that overlap across rings.
            xt00a = xtp.tile([128, 2, 512], BF16, tag="xt0a", name="xt_sb0a")
            nc.sync.dma_start(out=xt00a[:, :, :], in_=xt_r[:, 0:2, 0:subs[0]])

            # rest of w1, staged so each slice lands ahead of its fc group
            for lo, hi in ((128, 512), (512, 1024)):
                nc.sync.dma_start(out=w1_sb[:, :, lo:hi], in_=w1_r[:, :, lo:hi])

            # w2: first d-chunk unblocks L2(s0) (needed ~early with the
            # pipelined schedule), then the rest
            w2_sb = consts.tile([128, FC, D], BF16, tag="w2")
            nc.sync.dma_start(out=w2_sb[:, :, 0:128], in_=w2_r[:, :, 0:128])
            nc.sync.dma_start(out=w2_sb[:, :, 128:D], in_=w2_r[:, :, 128:D])

            # ---- scalar HWDGE ring: first x^T other half, b1, rest of x^T ----
            xt00b = xtp.tile([128, 2, 512], BF16, tag="xt0b", name="xt_sb0b")
            nc.scalar.dma_start(out=xt00b[:, :, :], in_=xt_r[:, 2:4, 0:subs[0]])

            b1_sb = consts.tile([128, FC], F32, tag="b1")
            nc.scalar.dma_start(out=b1_sb[:, :], in_=b1_r[:, :])

            xt_sub = {}

            def _load_xt(si, eng):
                soff, ssz = offs[si], subs[si]
                t = xtp.tile([128, DC, ssz], BF16, tag=f"xt{si}", name=f"xt_sb{si}")
                eng.dma_start(out=t[:, :, :], in_=xt_r[:, :, soff:soff + ssz])
                xt_sub[si] = t

            for si in range(1, nsub):
                _load_xt(si, nc.scalar)

            def layer1(si):
                ssz = subs[si]
                for fc in range(FC):
                    ps = ps_h.tile([128, ssz], F32, tag="psh")
                    for dc in range(DC):
                        if si == 0:
                            rhs = (xt00a[:, dc, 0:ssz] if dc < 2
                                   else xt00b[:, dc - 2, 0:ssz])
                        else:
                            rhs = xt_sub[si][:, dc, :]
                        nc.tensor.matmul(
                            ps[:, :],
                            w1_sb[:, dc, fc * 128:(fc + 1) * 128],
                            rhs,
                            start=(dc == 0),
                            stop=(dc == DC - 1),
                        )
                    h = hp.tile([128, ssz], BF16, tag="h")
                    nc.scalar.activation(
                        h[:, :], ps[:, :],
                        mybir.ActivationFunctionType.Gelu_apprx_tanh,
                        bias=b1_sb[:, fc:fc + 1],
                    )
                    h_tiles[(si, fc)] = h

            def layer2(si):
                soff, ssz = offs[si], subs[si]
                for dc in range(DC):
                    ps2 = ps_y.tile([128, ssz], F32, tag="psy")
                    for fc in range(FC):
                        nc.tensor.matmul(
                            ps2[:, :],
                            w2_sb[:, fc, dc * 128:(dc + 1) * 128],
                            h_tiles[(si, fc)][:, :],
                            start=(fc == 0),
                            stop=(fc == FC - 1),
                        )
                    # PSUM -> bf16 SBUF on the DVE; combine weights and b2
                    # are applied on the host during the scatter.
                    yout = ybigp.tile([128, ssz], BF16, tag="yout")
                    nc.vector.tensor_copy(out=yout[:, :], in_=ps2[:, :])
                    nc.scalar.dma_start(
                        out=yt_r[:, dc, soff:soff + ssz],
                        in_=yout[:, :],
                    )

            # software pipeline: L1 runs one sub ahead of L2, so outputs
            # drain throughout instead of bunching at the end
            h_tiles = {}
            layer1(0)
            for si in range(1, nsub):
                layer1(si)
                layer2(si - 1)
            layer2(nsub - 1)

    nc.finalize()
    return nc


def kernel(hidden, top_k_indices, top_k_weights, W1, b1, W2, b2):
    global LAST_RESULTS
    x = np.ascontiguousarray(np.asarray(hidden, dtype=np.float32).reshape(T, D))
    idx = np.asarray(top_k_indices).reshape(T, TOPK)
    w = np.asarray(top_k_weights, dtype=np.float32).reshape(T, TOPK)
    W1 = np.asarray(W1, dtype=np.float32)
    b1 = np.asarray(b1, dtype=np.float32)
    W2 = np.asarray(W2, dtype=np.float32)
    b2 = np.asarray(b2, dtype=np.float32)

    # Host routing: token lists + combined weights per expert
    tok_lists, cw_lists = [], []
    for e in range(E):
        m = idx == e
        toks = np.nonzero(m.any(axis=1))[0]
        cw_t = (w * m).sum(axis=1)[toks]
        tok_lists.append(toks)
        cw_lists.append(cw_t)

    maxn = max(len(t) for t in tok_lists)
    C = max(512, -(-maxn // 64) * 64)

    if C not in _nc_cache:
        _nc_cache[C] = _build_nc(C)
    nc = _nc_cache[C]

    in_maps = []
    for e in range(E):
        toks = tok_lists[e]
        n = len(toks)
        xt = np.zeros((D, C), NPBF16)
        xt[:, :n] = x[toks].T.astype(NPBF16)
        in_maps.append({
            "xt": xt,
            "w1": np.ascontiguousarray(W1[e].astype(NPBF16)),
            "b1": np.ascontiguousarray(b1[e].reshape(F, 1)),
            "w2": np.ascontiguousarray(W2[e].astype(NPBF16)),
        })

    kwargs = {}
    if TRACE:
        kwargs = dict(trace=True, trace_cores=list(range(E)))
    res = run_bass_kernel_spmd(nc, in_maps, core_ids=list(range(E)), **kwargs)
    LAST_RESULTS = res

    out = np.zeros((T, D), np.float32)
    for e in range(E):
        toks = tok_lists[e]
        n = len(toks)
        y = res.results[e]["yt"][:, :n].astype(np.float32).T
        out[toks] += cw_lists[e][:, None] * y
        if b2[e].any():
            out[toks] += cw_lists[e][:, None] * b2[e][None, :]
    return out.reshape(B, S, D)
